# revision 45
# baseline (speedup 1.0000x reference)
"""Trainium2 Bass kernel for dual-branch local+dilated windowed attention.

Problem: B=1, L=4096, D=512, H=8 heads (dh=64), window=+-256, dilation=4.
reference returns (out_local, out_dilated), each [1, L, D] fp32.

Sharding: sequence (L) sharded across 8 cores; each core owns 512 query rows
and loads a 1024-row key slice (256-row halo each side, zero-padded at the
sequence edges).  All weights are replicated, pre-transposed, and cast to
bf16 host-side with the rmsnorm gains (and the 1/sqrt(dh) score scale)
folded in.

On-chip pipeline per core (single NEFF, SPMD over 8 cores):
  1. rmsnorm(x): ACT square+accum -> Ln/Exp rstd (one table set with the
     attention exp); rstd folded into the PE transpose via a diagonal
     matrix rhs -> xhatT [D, KL] bf16.
  2. Q/K/V projections per branch (PE, bf16, fp32 PSUM accum); V is stored
     keys-on-partitions as [k, 8 heads x (64 dv | colmask)] so the key
     validity mask rides in column 64 and becomes the softmax denominator.
  3. scores^T = K^T_chunk x Q per 128-key chunk -> exp (ACT, psum->sbuf)
     -> banded {0,1} triangle mask multiplies (DVE, batched strided APs).
  4. attnV transposed: O^T[dv,q] = [V|cmask]^T @ ex accumulated over the
     band chunks straight into the Wo-ready layout; row 64 is the softmax
     denominator.
  5. denominators: evac carries them to SBUF row 64; DMA partition-shuffle
     -> reciprocal_approx_fast on 128 lanes -> DMA back -> PE broadcast to
     [64, q] -> one fused normalize multiply per head pair.
  6. Wo per (branch, qtile) over 4 head pairs (odd heads DMA-relocated to
     partitions 64-127), output DMA (dilated through a strided view).
"""

import numpy as np
import ml_dtypes

L, D, H, DH = 4096, 512, 8, 64
WIN, DIL = 256, 4
EPS = 1e-6
NCORES = 8
QL = L // NCORES          # 512 queries per core
KL = QL + 2 * WIN         # 1024 keys per core (halo)
P = 128
NKC = KL // P             # 8 key chunks
NQT = QL // P             # 4 query tiles
EXW = 640                 # per-chunk ex row width (max band cols per chunk)
BF16 = ml_dtypes.bfloat16

_STATE = {}


def _band(kc):
    """Query range [qlo, qhi) covered by local key chunk kc."""
    return max(0, P * (kc - 4)), min(QL, P * kc + P)


def _build_nc():
    import concourse.bacc as bacc
    import concourse.tile as tile
    import concourse.mybir as mybir
    from concourse.bass import broadcast_tensor_aps
    from concourse.masks import make_identity

    f32 = mybir.dt.float32
    bf16 = mybir.dt.bfloat16
    Exp = mybir.ActivationFunctionType.Exp
    Ln = mybir.ActivationFunctionType.Ln
    Square = mybir.ActivationFunctionType.Square
    Copy = mybir.ActivationFunctionType.Copy

    nc = bacc.Bacc()

    xn = nc.dram_tensor("xn", [KL, D], bf16, kind="ExternalInput")
    wT = {}
    for br in ("l", "d"):
        for w in ("wq", "wk", "wv", "wo"):
            wT[w, br] = nc.dram_tensor(f"{w}T_{br}", [D, D], bf16,
                                       kind="ExternalInput")
    tri_lo_d = nc.dram_tensor("tri_lo", [P, P], bf16, kind="ExternalInput")
    tri_hi_d = nc.dram_tensor("tri_hi", [P, P], bf16, kind="ExternalInput")
    colmask_d_ = {
        "l": nc.dram_tensor("colmask_l", [P, NKC], f32, kind="ExternalInput"),
        "d": nc.dram_tensor("colmask_d", [P, NKC], f32, kind="ExternalInput"),
    }
    cm8_d_ = {
        "l": nc.dram_tensor("cm8_l", [P, NKC, H], bf16, kind="ExternalInput"),
        "d": nc.dram_tensor("cm8_d", [P, NKC, H], bf16, kind="ExternalInput"),
    }
    out_dram = {
        "l": nc.dram_tensor("out_l", [QL, D], f32, kind="ExternalOutput"),
        "d": nc.dram_tensor("out_d", [QL, D], f32, kind="ExternalOutput"),
    }

    with tile.TileContext(nc) as tc:
        with (
            tc.tile_pool(name="singles", bufs=1) as singles,
            tc.tile_pool(name="xpool", bufs=3) as xpool,
            tc.tile_pool(name="small", bufs=4) as small,
            tc.tile_pool(name="expl", bufs=3) as expl,
            tc.tile_pool(name="expd", bufs=3) as expd,
            tc.tile_pool(name="ottmp", bufs=4) as ottmp,
            tc.tile_pool(name="rcpp", bufs=2) as rcpp,
            tc.tile_pool(name="outpool", bufs=2) as outpool,
            tc.tile_pool(name="ptr", bufs=1, space="PSUM") as psum_tr,
            tc.tile_pool(name="pproj", bufs=2, space="PSUM") as psum_proj,
            tc.tile_pool(name="pst", bufs=2, space="PSUM") as psum_st,
            tc.tile_pool(name="po", bufs=2, space="PSUM") as psum_o,
            tc.tile_pool(name="pbc", bufs=1, space="PSUM") as psum_bc,
        ):
            identity = singles.tile([P, P], bf16)
            make_identity(nc, identity)
            xhatT = singles.tile([P, 4, KL], bf16, name="xhatT")
            eps_t = singles.tile([P, 1], f32, name="eps")
            nc.vector.memset(eps_t, EPS)
            eps_bc4 = singles.tile([P, 4], f32, name="eps_bc4")
            nc.vector.memset(eps_bc4, EPS)
            ones64 = singles.tile([1, DH], bf16, name="ones64")
            nc.vector.memset(ones64, 1.0)
            ones65 = singles.tile([1, DH + 1], bf16, name="ones65")
            nc.vector.memset(ones65, 1.0)
            zrow = singles.tile([1, QL], bf16, name="zrow")
            nc.vector.memset(zrow, 0.0)

            # x first, as two big DMAs (one per HWDGE queue): the per-DMA
            # fixed cost (~2us) dominates 128KB transfers, so batch them.
            xsb = singles.tile([P, NKC, D], bf16, name="xsb")
            for qt in range(4):
                eng = nc.sync if qt % 2 == 0 else nc.scalar
                eng.dma_start(
                    xsb[:, 2 * qt:2 * (qt + 1), :],
                    xn[qt * 2 * P:(qt + 1) * 2 * P, :].rearrange(
                        "(t p) d -> p t d", p=P))
            xts = [xsb[:, tt, :] for tt in range(NKC)]
            # weights ride the two HWDGE FIFO queues behind the x tiles so
            # x gets the SDMA bandwidth first; order = first-needed first.
            w_sb = {}
            w_order = [("wq", "l"), ("wk", "l"), ("wq", "d"), ("wk", "d"),
                       ("wv", "l"), ("wv", "d"), ("wo", "l"), ("wo", "d")]
            for i, (w, br) in enumerate(w_order):
                w_sb[w, br] = singles.tile([P, 4, D], bf16, name=f"{w}_{br}")
                dma_eng = nc.scalar if i % 2 == 0 else nc.sync
                dma_eng.dma_start(
                    w_sb[w, br],
                    wT[w, br][:, :].rearrange("(ic p) o -> p ic o", p=P),
                )
            tri_lo = singles.tile([P, P], bf16)
            nc.gpsimd.dma_start(tri_lo, tri_lo_d[:, :])
            tri_hi = singles.tile([P, P], bf16)
            nc.gpsimd.dma_start(tri_hi, tri_hi_d[:, :])
            colmask, cm8 = {}, {}
            for br in ("l", "d"):
                colmask[br] = singles.tile([P, NKC], f32, name=f"cm_{br}")
                nc.gpsimd.dma_start(colmask[br], colmask_d_[br][:, :])
                cm8[br] = singles.tile([P, NKC, H], bf16, name=f"cm8_{br}")
                nc.gpsimd.dma_start(cm8[br], cm8_d_[br][:, :, :])

            # ---- rmsnorm + transpose (rstd folded into a diagonal rhs).
            # sum(x^2) split DVE/ACT per half; rstd = rsqrt(ms) via Newton
            # on DVE (ms = mean(x^2)+eps is ~1 so y0=1 converges in 3
            # iters) -- keeps ACT table loads off the critical path.
            Mult = mybir.AluOpType.mult
            Add = mybir.AluOpType.add
            ssums = small.tile([P, NKC], f32, name="ssums")
            rstds = small.tile([P, NKC], f32, name="rstds")
            scr = small.tile([P, NKC, 2], f32, name="scr")
            for hf in range(2):
                for j in range(4):
                    tt = 4 * hf + j
                    sqd2 = xpool.tile([P, D], f32, tag="sqd2")
                    nc.scalar.activation(sqd2, xts[tt], Square,
                                         accum_out=ssums[:, tt:tt + 1])
                ms = scr[:, 4 * hf:4 * (hf + 1), 0]
                nc.vector.scalar_tensor_tensor(
                    ms, ssums[:, 4 * hf:4 * (hf + 1)], 1.0 / D, eps_bc4,
                    Mult, Add)
                y = rstds[:, 4 * hf:4 * (hf + 1)]
                nc.vector.memset(y, 1.0)
                for _ in range(3):
                    y2 = scr[:, 4 * hf:4 * (hf + 1), 1]
                    nc.vector.scalar_tensor_tensor(y2, y, 1.0, y, Mult, Mult)
                    nc.vector.scalar_tensor_tensor(y2, y2, -0.5, ms,
                                                   Mult, Mult)
                    nc.vector.scalar_tensor_tensor(y, y2, 1.5, y, Add, Mult)
            for tt in range(NKC):
                diag = xpool.tile([P, P], bf16, tag="diag")
                nc.vector.tensor_scalar_mul(diag, identity,
                                            rstds[:, tt:tt + 1])
                tp = psum_tr.tile([P, D], f32, tag="tp")
                for ic in range(4):
                    nc.tensor.matmul(tp[:, ic * P:(ic + 1) * P],
                                     xts[tt][:, ic * P:(ic + 1) * P], diag)
                nc.vector.tensor_copy(
                    xhatT[:, :, tt * P:(tt + 1) * P],
                    tp.rearrange("p (ic q) -> p ic q", ic=4))

            QT, KT, V, OT = {}, {}, {}, {}
            for br in ("l", "d"):
                QT[br] = singles.tile([P, 4, QL], bf16, name=f"QT_{br}")
                KT[br] = singles.tile([P, 4, KL], bf16, name=f"KT_{br}")
                V[br] = singles.tile([P, NKC, H, DH + 1], bf16, name=f"V_{br}")
                OT[br] = singles.tile([P, 4, QL], bf16, name=f"OT_{br}")

            def key_cols_ap(ic, kc, br):
                # lhsT [128, 128] of xhat^T columns for key chunk kc
                if br == "l":
                    return xhatT[:, ic, kc * P:(kc + 1) * P]
                rho, s = kc // 2, kc % 2
                return xhatT[:, ic, :].rearrange(
                    "p (b four) -> p four b", four=DIL)[:, rho, s * P:(s + 1) * P]

            # ---- projections (3rd psum bank borrowed from the idle
            # broadcast pool to keep PE fed while evacs drain) ----
            pidx = [0]

            def proj_ps():
                pidx[0] += 1
                if pidx[0] % 3 == 0:
                    return psum_bc.tile([P, QL], f32, tag="bc", name="psb")
                return psum_proj.tile([P, D], f32, tag="pp", name="psp")

            for br in ("l", "d"):
                # ones (=colmask) columns of V, one strided copy per branch
                nc.vector.tensor_copy(
                    V[br][:, :, :, DH:DH + 1].rearrange("p a h o -> p a (h o)"),
                    cm8[br][:, :, :])
                for pair in range(4):
                    ps = proj_ps()
                    for ic in range(4):
                        nc.tensor.matmul(
                            ps, w_sb["wq", br][:, ic, pair * P:(pair + 1) * P],
                            xhatT[:, ic, WIN:WIN + QL],
                            start=(ic == 0), stop=(ic == 3))
                    nc.vector.tensor_copy(QT[br][:, pair, :], ps)
                for pair in range(4):
                    for half in range(2):
                        ps = proj_ps()
                        for ic in range(4):
                            nc.tensor.matmul(
                                ps, w_sb["wk", br][:, ic, pair * P:(pair + 1) * P],
                                xhatT[:, ic, half * D:(half + 1) * D],
                                start=(ic == 0), stop=(ic == 3))
                        nc.vector.tensor_copy(
                            KT[br][:, pair, half * D:(half + 1) * D], ps)
                for kc in range(NKC):
                    ps = proj_ps()
                    for ic in range(4):
                        nc.tensor.matmul(
                            ps, key_cols_ap(ic, kc, br), w_sb["wv", br][:, ic, :],
                            start=(ic == 0), stop=(ic == 3))
                    # evac with the key-validity mask folded in (per-partition)
                    eng = nc.scalar if kc % 2 == 0 else nc.vector
                    if kc % 2 == 0:
                        nc.scalar.activation(
                            V[br][:, kc, :, 0:DH],
                            ps.rearrange("p (h dv) -> p h dv", h=H),
                            Copy, scale=colmask[br][:, kc:kc + 1])
                    else:
                        nc.vector.tensor_scalar_mul(
                            V[br][:, kc, :, 0:DH],
                            ps.rearrange("p (h dv) -> p h dv", h=H),
                            colmask[br][:, kc:kc + 1])

            # ---- attention ----
            # per head: scores^T per key chunk -> exp -> triangle masks ->
            # O^T = [V | cmask]^T @ ex accumulated over the band (row 64 =
            # softmax denominator).  Head-even lands on partitions 0-64 of
            # the pair tile, head-odd is DMA-relocated to partitions 64-127.
            gath = singles.tile([P, 2, H, 4], bf16, name="gath")
            ibr = {"l": 0, "d": 1}

            def scores_head(br, h):
                r0, pair = DH * (h % 2), h // 2
                if br == "l":
                    ex = expl.tile([P, NKC, EXW], bf16, tag="exl")
                    for kc in range(NKC):
                        qlo, qhi = _band(kc)
                        n = qhi - qlo
                        # third score bank: reuse the idle transpose pool
                        if kc % 3 == 2:
                            st = psum_tr.tile([P, QL], f32, tag="tp")
                        else:
                            st = psum_st.tile([P, QL], f32, tag="st")
                        nc.tensor.matmul(
                            st[:, :n],
                            KT[br][r0:r0 + 64, pair, kc * P:(kc + 1) * P],
                            QT[br][r0:r0 + 64, pair, qlo:qhi])
                        nc.scalar.activation(ex[:, kc, 0:n], st[:, :n], Exp)
                    # triangle masks, batched: chunks 0-3 tri_lo at local
                    # offset 128*kc (stride EXW+128); chunks 4-7 tri_hi at 0.
                    exf = ex.rearrange("p a b -> p (a b)")
                    g1 = exf[:, 0:4 * (EXW + P)].rearrange(
                        "p (a c) -> p a c", c=EXW + P)[:, :, 0:P]
                    g2 = exf[:, 4 * EXW:8 * EXW].rearrange(
                        "p (a c) -> p a c", c=EXW)[:, :, 0:P]
                    for g, tri in ((g1, tri_lo), (g2, tri_hi)):
                        ga, ta = broadcast_tensor_aps(
                            g, tri[:, :].rearrange("p (o b) -> p o b", o=1))
                        nc.vector.tensor_mul(ga, ga, ta)
                else:
                    ex = expd.tile([P, NKC, P], bf16, tag="exd")
                    for half in range(2):
                        st = psum_st.tile([P, QL], f32, tag="st")
                        for j in range(4):
                            idx = half * 4 + j
                            rho, s = idx // 2, idx % 2
                            ktv = KT[br][r0:r0 + 64, pair, :].rearrange(
                                "p (b four) -> p four b", four=DIL
                            )[:, rho, s * P:(s + 1) * P]
                            qtv = QT[br][r0:r0 + 64, pair, :].rearrange(
                                "p (a four) -> p four a", four=DIL)[:, rho, :]
                            nc.tensor.matmul(st[:, j * P:(j + 1) * P], ktv, qtv)
                        nc.scalar.activation(
                            ex[:, half * 4:(half + 1) * 4, :], st, Exp)
                    exg = ex.rearrange("p (a two) b -> p a two b", two=2)
                    for s, tri in ((0, tri_lo), (1, tri_hi)):
                        g = exg[:, :, s, :]
                        ga, ta = broadcast_tensor_aps(
                            g, tri[:, :].rearrange("p (o b) -> p o b", o=1))
                        nc.vector.tensor_mul(ga, ga, ta)
                return ex

            def av_head(br, h, ex):
                r0, pair = DH * (h % 2), h // 2
                if br == "l":
                    # chunks 4 and 3 span all 512 queries: open the group
                    # with kc=4 (whole-bank start) and close it with kc=3.
                    op = psum_o.tile([DH + 1, QL], f32, tag="op")
                    for kc in (4, 0, 1, 2, 5, 6, 7, 3):
                        qlo, qhi = _band(kc)
                        nc.tensor.matmul(
                            op[:, qlo:qhi], V[br][:, kc, h, :],
                            ex[:, kc, 0:qhi - qlo],
                            start=(kc == 4), stop=(kc == 3),
                            skip_group_check=True)
                else:
                    # whole-bank has_written init via a K=1 zero matmul,
                    # then accumulate all 8 (rho, s) chunks.
                    op = psum_o.tile([DH + 1, QL], f32, tag="op")
                    nc.tensor.matmul(op, ones65, zrow, start=True, stop=False,
                                     skip_group_check=True)
                    for rho in range(DIL):
                        for s in range(2):
                            nc.tensor.matmul(
                                op[:, rho * P:(rho + 1) * P],
                                V[br][:, rho * 2 + s, h, :],
                                ex[:, rho * 2 + s, :],
                                start=False, stop=False,
                                skip_group_check=True)
                    nc.tensor.matmul(op, ones65, zrow, start=False, stop=True,
                                     skip_group_check=True)
                # evacuate rows 0-64 (65th row = unnormalized denominator)
                if h % 2 == 0:
                    dst = OT[br][0:DH + 1, pair, :]
                    nc.vector.tensor_copy(dst, op)
                else:
                    tmp = ottmp.tile([DH + 1, QL], bf16, tag="ot")
                    nc.vector.tensor_copy(tmp, op)
                # gather denominator row onto 128 lanes: gath[p, a] = den[4p+a]
                src = OT[br][DH:DH + 1, pair, :] if h % 2 == 0 \
                    else tmp[DH:DH + 1, :]
                nc.gpsimd.dma_start(gath[:, ibr[br], h, :], src)
                return None if h % 2 == 0 else tmp

            def normalize_pair(br, pair, rcq, tmp_odd):
                # relocate odd head to partitions 64-127 (overwrites den row)
                nc.gpsimd.dma_start(OT[br][DH:P, pair, :], tmp_odd[0:DH, :])
                # broadcast 1/den rows across partitions via K=1 matmuls
                bc = psum_bc.tile([P, QL], f32, tag="bc")
                nc.tensor.matmul(bc[0:DH, :], ones64, rcq[2 * pair])
                nc.tensor.matmul(bc[DH:P, :], ones64, rcq[2 * pair + 1],
                                 tile_position=(0, 64))
                nc.vector.tensor_mul(OT[br][:, pair, :], OT[br][:, pair, :], bc)

            def wo_out(br, t):
                # t is a query tile for local, a residue class for dilated
                ps = psum_proj.tile([P, D], f32, tag="pp")
                for pair in range(4):
                    lhs = OT[br][:, pair, t * P:(t + 1) * P]
                    nc.tensor.matmul(ps, lhs, w_sb["wo", br][:, pair, :],
                                     start=(pair == 0), stop=(pair == 3))
                ob = outpool.tile([P, D], f32, tag="ob")
                if t % 2 == 0:
                    nc.scalar.copy(ob, ps)
                else:
                    nc.vector.tensor_copy(ob, ps)
                if br == "l":
                    nc.sync.dma_start(out_dram[br][t * P:(t + 1) * P, :], ob)
                else:
                    dst = out_dram[br][:, :].rearrange(
                        "(a four) o -> four a o", four=DIL)[t]
                    nc.sync.dma_start(dst, ob)

            def den_recip(br):
                # 1/den for all 8 heads of a branch on 128 lanes
                gathf = small.tile([P, H * 4], f32, tag="gf")
                nc.vector.tensor_copy(
                    gathf, gath[:, ibr[br], :, :].rearrange("p h a -> p (h a)"))
                rc = small.tile([P, H * 4], f32, tag="rc")
                nc.vector.reciprocal_approx_fast(rc, gathf)
                # one batched cast-DMA back (contiguous): rows[32p+4h+a]
                rows = rcpp.tile([1, H * QL], bf16, tag="rq")
                nc.gpsimd.dma_start(rows, rc)
                # per-head view with q = 4p + a  ->  offset 32p + 4h + a
                rv = rows.rearrange("o (p h a) -> o p h a", p=P, h=H)
                return [rv[:, :, h, :] for h in range(H)]

            # software-pipeline: emit head h+1's scores before head h's
            # attnV so the PE never stalls on the exp of the current head.
            tmp_odd = {}
            for br in ("l", "d"):
                exs = [scores_head(br, 0), scores_head(br, 1)]
                for h in range(H):
                    if h + 2 < H:
                        exs.append(scores_head(br, h + 2))
                    r = av_head(br, h, exs[h])
                    if r is not None:
                        tmp_odd[br, h] = r
                rcq = den_recip(br)
                for pair in range(4):
                    normalize_pair(br, pair, rcq, tmp_odd[br, 2 * pair + 1])
                for t in range(NQT):
                    wo_out(br, t)

    nc.finalize()
    return nc


def _prep_host(x, key_padding_mask, weights):
    """Build the per-core input maps (weights shared across cores)."""
    x = np.asarray(x, dtype=np.float32).reshape(L, D)
    kpm = np.asarray(key_padding_mask).reshape(L).astype(bool)

    shared = {}
    for name, arr in weights.items():
        shared[name] = np.ascontiguousarray(arr.T).astype(BF16)

    idx = np.arange(P)
    tri_lo = (idx[:, None] >= idx[None, :]).astype(BF16)
    tri_hi = (idx[:, None] <= idx[None, :]).astype(BF16)
    shared["tri_lo"], shared["tri_hi"] = tri_lo, tri_hi

    valid_full = np.zeros(L + 2 * WIN, dtype=np.float32)
    valid_full[WIN:WIN + L] = (~kpm).astype(np.float32)

    in_maps = []
    for c in range(NCORES):
        lo = c * QL - WIN
        xnc = np.zeros((KL, D), dtype=np.float32)
        a, b = max(lo, 0), min(lo + KL, L)
        xnc[a - lo:b - lo] = x[a:b]
        v = valid_full[lo + WIN:lo + WIN + KL]  # validity of keys lo..lo+KL
        cm_l = v.reshape(NKC, P).T.astype(np.float32)
        # dilated chunk idx = rho*2+s holds keys lk = 4*(128*s + p) + rho
        cm_d = np.empty((P, NKC), dtype=np.float32)
        for rho in range(DIL):
            for s in range(2):
                lk = DIL * (P * s + idx) + rho
                cm_d[:, rho * 2 + s] = v[lk]
        m = dict(shared)
        m["xn"] = xnc.astype(BF16)
        m["colmask_l"] = np.ascontiguousarray(cm_l)
        m["colmask_d"] = np.ascontiguousarray(cm_d)
        m["cm8_l"] = np.ascontiguousarray(
            np.repeat(cm_l[:, :, None], H, axis=2)).astype(BF16)
        m["cm8_d"] = np.ascontiguousarray(
            np.repeat(cm_d[:, :, None], H, axis=2)).astype(BF16)
        in_maps.append(m)
    return in_maps


def kernel(x, key_padding_mask, wq_l, wk_l, wv_l, wo_l,
           wq_d, wk_d, wv_d, wo_d, g_q, g_kv, **run_kwargs):
    from concourse.bass_utils import run_bass_kernel_spmd

    g_q = np.asarray(g_q, dtype=np.float32)
    g_kv = np.asarray(g_kv, dtype=np.float32)
    scale = 1.0 / np.sqrt(DH)
    weights = {
        "wqT_l": np.asarray(wq_l, np.float32) * (g_q * scale)[None, :],
        "wkT_l": np.asarray(wk_l, np.float32) * g_kv[None, :],
        "wvT_l": np.asarray(wv_l, np.float32) * g_kv[None, :],
        "woT_l": np.asarray(wo_l, np.float32),
        "wqT_d": np.asarray(wq_d, np.float32) * (g_q * scale)[None, :],
        "wkT_d": np.asarray(wk_d, np.float32) * g_kv[None, :],
        "wvT_d": np.asarray(wv_d, np.float32) * g_kv[None, :],
        "woT_d": np.asarray(wo_d, np.float32),
    }
    in_maps = _prep_host(x, key_padding_mask, weights)

    if "nc" not in _STATE:
        _STATE["nc"] = _build_nc()
    res = run_bass_kernel_spmd(_STATE["nc"], in_maps,
                               core_ids=list(range(NCORES)), **run_kwargs)
    _STATE["last_result"] = res

    out_l = np.concatenate([res.results[c]["out_l"] for c in range(NCORES)],
                           axis=0).reshape(1, L, D)
    out_d = np.concatenate([res.results[c]["out_d"] for c in range(NCORES)],
                           axis=0).reshape(1, L, D)
    return (out_l, out_d)
